# revision 18
# baseline (speedup 1.0000x reference)
"""MoE router kernel for Trainium2 (8 NeuronCores, token-dim sharding).

Computes, for full inputs x[16384,2048], w_gate[2048,64], noise[16384,64]:
  scores  = x @ w_gate
  probs   = softmax(scores + noise)       (never materialized -- cancels)
  top-2   values/indices per token
  gates   = renormalized top-2 probs  = sigmoid(+/-(s1 - s2))
  combine = gates scattered into [T, E]
  expert_activation = per-expert selection counts

Sharding / layout strategy (host side, inside kernel()):
  - token dim T split 8 ways (2048 tokens/core); w_gate replicated
  - x is shipped in a 2x-bf16 "pseudo-fp32" format, pre-transposed per
    shard: xthi = bf16(x^T), xtlo = bf16(x^T - xthi). Same total bytes as
    fp32 x (16 MB/core); scores = xhi@whi + xhi@wlo + xlo@whi carries
    ~2^-17 relative error (verified: 0 top-2 flips vs the fp32 reference
    on the actual dataset). w_gate is split the same way.
  - expert_activation partial counts summed on host after gather.

Device kernel, streamed per 512-token block (DMA overlaps PE):
  - load the block's transposed x planes (4 DMAs, 1 MB each)
  - PE accumulates scoresT[e,t] via 48 bf16 matmuls (3 plane-terms x 16
    contraction chunks, 1 cyc/row moving operand, N=512)
  - ACT copies scoresT PSUM->SBUF; PE back-transposes 128-token chunks
    to scores[t,e] via identity matmul
  - DVE adds noise, max8 + max_index give top-2 vals/idx
  - gates via sigmoid(+/-(v1-v2)) on ACT (softmax denominator cancels)
  - combine via compare-masks against v1/v2 (no scatter); the g2 scaling
    runs on ACT to balance DVE
  - counts via ones^T @ (s >= v2) matmul accumulated in PSUM
  - outputs staged per block, written via the Scalar HWDGE queue
"""

from contextlib import ExitStack

import ml_dtypes
import numpy as np

import concourse.bass as bass
import concourse.mybir as mybir
import concourse.tile as tile
from concourse import bacc
from concourse.bass_utils import run_bass_kernel_spmd
from concourse.masks import make_identity

N_CORES = 8
T, D, E = 16384, 2048, 64
TS = T // N_CORES          # tokens per core
P = 128                    # partitions
TB = 512                   # token block (psum scoresT width)
N_TB = TS // TB            # 4 token blocks per core
N_DC = D // P              # 16 contraction chunks
HALF = N_DC // 2           # d-chunks per load half
CHUNKS_PER_TB = TB // P    # 4

FP32 = mybir.dt.float32
BF16 = mybir.dt.bfloat16
U32 = mybir.dt.uint32

BF16_NP = ml_dtypes.bfloat16


def _build_bass():
    nc = bacc.Bacc(trn_type="TRN2", debug=False, num_devices=N_CORES)

    xthi_d = nc.dram_tensor("xthi", [D, TS], BF16, kind="ExternalInput").ap()
    xtlo_d = nc.dram_tensor("xtlo", [D, TS], BF16, kind="ExternalInput").ap()
    whi_d = nc.dram_tensor("whi", [D, E], BF16, kind="ExternalInput").ap()
    wlo_d = nc.dram_tensor("wlo", [D, E], BF16, kind="ExternalInput").ap()
    noise_d = nc.dram_tensor("noise", [TS, E], FP32, kind="ExternalInput").ap()

    combine_d = nc.dram_tensor("combine", [TS, E], FP32, kind="ExternalOutput").ap()
    # misc packs [g1, g2, idx1(bits), idx2(bits)] per token
    misc_d = nc.dram_tensor("misc", [TS, 4], FP32, kind="ExternalOutput").ap()
    cnt_d = nc.dram_tensor("cnt", [1, E], FP32, kind="ExternalOutput").ap()

    with tile.TileContext(nc) as tc, ExitStack() as ctx:
        consts = ctx.enter_context(tc.tile_pool(name="consts", bufs=1))
        xin = ctx.enter_context(tc.tile_pool(name="xin", bufs=3))
        stsb = ctx.enter_context(tc.tile_pool(name="stsb", bufs=2))
        small = ctx.enter_context(tc.tile_pool(name="small", bufs=3))
        outp = ctx.enter_context(tc.tile_pool(name="outp", bufs=2))
        psum_st = ctx.enter_context(tc.tile_pool(name="psum_st", bufs=2, space="PSUM"))
        psum_s = ctx.enter_context(tc.tile_pool(name="psum_s", bufs=4, space="PSUM"))
        psum_c = ctx.enter_context(tc.tile_pool(name="psum_c", bufs=1, space="PSUM"))

        identity = consts.tile([P, P], FP32)
        make_identity(nc, identity)
        ones = consts.tile([P, 1], FP32)
        nc.vector.memset(ones, 1.0)

        # w planes as [128 d-in-chunk, 16 d-chunks, 64 experts]
        whi_sb = consts.tile([P, N_DC, E], BF16)
        nc.gpsimd.dma_start(out=whi_sb, in_=whi_d.rearrange("(c p) e -> p c e", p=P))
        wlo_sb = consts.tile([P, N_DC, E], BF16)
        nc.gpsimd.dma_start(out=wlo_sb, in_=wlo_d.rearrange("(c p) e -> p c e", p=P))

        # all noise for the shard: [128, 16 chunks, 64]
        noise_sb = consts.tile([P, TS // P, E], FP32)
        nc.gpsimd.dma_start(
            out=noise_sb, in_=noise_d.rearrange("(c p) e -> p c e", p=P)
        )

        cnt_ps = psum_c.tile([1, E], FP32)

        for tb in range(N_TB):
            cols = slice(tb * TB, (tb + 1) * TB)

            # block's transposed x planes, split in d-halves so matmuls can
            # start after the first quarter of the block's data
            halves = []
            for h in range(2):
                hrows = slice(h * HALF * P, (h + 1) * HALF * P)
                th = xin.tile([P, HALF, TB], BF16, tag=f"xh{h}")
                nc.sync.dma_start(
                    out=th,
                    in_=xthi_d[hrows, cols].rearrange("(c p) t -> p c t", p=P),
                )
                tl = xin.tile([P, HALF, TB], BF16, tag=f"xl{h}")
                nc.scalar.dma_start(
                    out=tl,
                    in_=xtlo_d[hrows, cols].rearrange("(c p) t -> p c t", p=P),
                )
                halves.append((th, tl))

            st_ps = psum_st.tile([E, TB], FP32)
            for dc in range(N_DC):
                h, c = dc // HALF, dc % HALF
                rhs_hi = halves[h][0][:, c, :]
                rhs_lo = halves[h][1][:, c, :]
                nc.tensor.matmul(
                    st_ps, lhsT=whi_sb[:, dc, :], rhs=rhs_hi,
                    start=(dc == 0), stop=False, skip_group_check=True,
                )
                nc.tensor.matmul(
                    st_ps, lhsT=whi_sb[:, dc, :], rhs=rhs_lo,
                    start=False, stop=False, skip_group_check=True,
                )
                nc.tensor.matmul(
                    st_ps, lhsT=wlo_sb[:, dc, :], rhs=rhs_hi,
                    start=False, stop=(dc == N_DC - 1), skip_group_check=True,
                )

            st_sb = stsb.tile([E, TB], FP32)
            for k in range(CHUNKS_PER_TB):
                ksl = slice(k * P, (k + 1) * P)
                if k % 2 == 0:
                    nc.scalar.copy(st_sb[:, ksl], st_ps[:, ksl])
                else:
                    nc.vector.tensor_copy(st_sb[:, ksl], st_ps[:, ksl])

            comb_tb = outp.tile([P, CHUNKS_PER_TB, E], FP32)
            misc_tb = outp.tile([P, CHUNKS_PER_TB, 4], FP32)

            for k in range(CHUNKS_PER_TB):
                tcv = tb * CHUNKS_PER_TB + k

                score_ps = psum_s.tile([P, E], FP32)
                nc.tensor.transpose(
                    score_ps, st_sb[:, k * P : (k + 1) * P], identity[0:E, 0:E]
                )

                # s = scores + noise
                s_t = small.tile([P, E], FP32)
                nc.vector.tensor_add(s_t, score_ps, noise_sb[:, tcv, :])

                top8 = small.tile([P, 8], FP32)
                nc.vector.max(out=top8, in_=s_t)
                idx8 = small.tile([P, 8], U32)
                nc.vector.max_index(out=idx8, in_max=top8, in_values=s_t)

                dv = small.tile([P, 1], FP32)
                nc.vector.tensor_sub(dv, top8[:, 0:1], top8[:, 1:2])

                misc_k = misc_tb[:, k, :]
                nc.scalar.activation(
                    misc_k[:, 0:1], dv, mybir.ActivationFunctionType.Sigmoid,
                    bias=0.0, scale=1.0,
                )
                nc.scalar.activation(
                    misc_k[:, 1:2], dv, mybir.ActivationFunctionType.Sigmoid,
                    bias=0.0, scale=-1.0,
                )
                nc.gpsimd.tensor_copy(misc_k[:, 2:4].bitcast(U32), idx8[:, 0:2])

                cmp2 = small.tile([P, E], FP32)
                nc.vector.tensor_scalar(
                    cmp2, s_t, top8[:, 1:2], None, op0=mybir.AluOpType.is_ge
                )
                cmp1 = small.tile([P, E], U32)
                nc.vector.tensor_scalar(
                    cmp1, s_t, top8[:, 0:1], None, op0=mybir.AluOpType.is_ge
                )

                # combine = cmp2 * g2, then g1 where cmp1
                comb_k = comb_tb[:, k, :]
                nc.scalar.mul(comb_k, cmp2, misc_k[:, 1:2])
                nc.vector.copy_predicated(
                    comb_k, cmp1, misc_k[:, 0:1].to_broadcast([P, E])
                )

                # expert activation counts: ones^T @ cmp2 accumulated over chunks
                nc.tensor.matmul(
                    cnt_ps, lhsT=ones, rhs=cmp2,
                    start=(tcv == 0), stop=(tcv == TS // P - 1),
                    skip_group_check=True,
                )

            nc.gpsimd.dma_start(
                out=combine_d[tb * TB : (tb + 1) * TB, :].rearrange(
                    "(c p) e -> p c e", p=P
                ),
                in_=comb_tb,
            )
            nc.gpsimd.dma_start(
                out=misc_d[tb * TB : (tb + 1) * TB, :].rearrange(
                    "(c p) k -> p c k", p=P
                ),
                in_=misc_tb,
            )

        cnt_sb = consts.tile([1, E], FP32)
        nc.vector.tensor_copy(cnt_sb, cnt_ps)
        nc.gpsimd.dma_start(out=cnt_d, in_=cnt_sb)

    nc.compile()
    return nc


_NC_CACHE = None


def kernel(x: np.ndarray, w_gate: np.ndarray, noise: np.ndarray, _trace=False):
    global _NC_CACHE
    x = np.ascontiguousarray(x, dtype=np.float32)
    w_gate = np.ascontiguousarray(w_gate, dtype=np.float32)
    noise = np.ascontiguousarray(noise, dtype=np.float32)

    # split into bf16 hi/lo planes (pseudo-fp32: hi + lo carries ~16-17
    # mantissa bits), then transpose each shard for the device layout
    xhi = x.astype(BF16_NP)
    xlo = (x - xhi.astype(np.float32)).astype(BF16_NP)
    whi = w_gate.astype(BF16_NP)
    wlo = (w_gate - whi.astype(np.float32)).astype(BF16_NP)

    in_maps = []
    for c in range(N_CORES):
        sh = slice(c * TS, (c + 1) * TS)
        in_maps.append(
            {
                "xthi": np.ascontiguousarray(xhi[sh].T),
                "xtlo": np.ascontiguousarray(xlo[sh].T),
                "whi": whi,
                "wlo": wlo,
                "noise": np.ascontiguousarray(noise[sh]),
            }
        )

    if _NC_CACHE is None:
        _NC_CACHE = _build_bass()

    res = run_bass_kernel_spmd(
        _NC_CACHE, in_maps, core_ids=list(range(N_CORES)), trace=_trace
    )

    combine = np.concatenate([res.results[c]["combine"] for c in range(N_CORES)], 0)
    misc = np.concatenate([res.results[c]["misc"] for c in range(N_CORES)], 0)
    gates = np.ascontiguousarray(misc[:, 0:2])
    topk_idx = np.ascontiguousarray(misc[:, 2:4]).view(np.int32)
    expert_activation = np.sum(
        [res.results[c]["cnt"][0] for c in range(N_CORES)], axis=0, dtype=np.float32
    )

    if _trace:
        kernel._last_results = res
    return combine, topk_idx, gates, expert_activation


# revision 21
# speedup vs baseline: 1.2252x; 1.2252x over previous
"""MoE router kernel for Trainium2 (8 NeuronCores, token-dim sharding).

Computes, for full inputs x[16384,2048], w_gate[2048,64], noise[16384,64]:
  scores  = x @ w_gate
  probs   = softmax(scores + noise)       (never materialized -- cancels)
  top-2   values/indices per token
  gates   = renormalized top-2 probs  = sigmoid(+/-(s1 - s2))
  combine = gates scattered into [T, E]
  expert_activation = per-expert selection counts

Sharding / layout strategy (host side, inside kernel()):
  - token dim T split 8 ways (2048 tokens/core); w_gate replicated
  - x is shipped in a 2x-bf16 "pseudo-fp32" format, pre-transposed per
    shard: xthi = bf16(x^T), xtlo = bf16(x^T - xthi). Same total bytes as
    fp32 x (16 MB/core); scores = xhi@whi + xhi@wlo + xlo@whi carries
    ~2^-17 relative error (verified: 0 top-2 flips vs the fp32 reference
    on the actual dataset). w_gate is split the same way.
  - expert_activation partial counts summed on host after gather.

Device kernel, streamed per 512-token block (DMA overlaps PE):
  - load the block's transposed x planes (4 DMAs, 1 MB each)
  - PE accumulates scoresT[e,t] via 48 bf16 matmuls (3 plane-terms x 16
    contraction chunks, 1 cyc/row moving operand, N=512)
  - ACT copies scoresT PSUM->SBUF; PE back-transposes 128-token chunks
    to scores[t,e] via identity matmul
  - DVE adds noise, max8 + max_index give top-2 vals/idx
  - gates via sigmoid(+/-(v1-v2)) on ACT (softmax denominator cancels)
  - combine via compare-masks against v1/v2 (no scatter); the g2 scaling
    runs on ACT to balance DVE
  - counts via ones^T @ (s >= v2) matmul accumulated in PSUM
  - outputs staged per block, written via the Scalar HWDGE queue
"""

from contextlib import ExitStack

import ml_dtypes
import numpy as np

import concourse.bass as bass
import concourse.mybir as mybir
import concourse.tile as tile
from concourse import bacc
from concourse.bass_utils import run_bass_kernel_spmd
from concourse.masks import make_identity

N_CORES = 8
T, D, E = 16384, 2048, 64
TS = T // N_CORES          # tokens per core
P = 128                    # partitions
TB = 512                   # token block (psum scoresT width)
N_TB = TS // TB            # 4 token blocks per core
N_DC = D // P              # 16 contraction chunks
HALF = N_DC // 2           # d-chunks per load half
CHUNKS_PER_TB = TB // P    # 4

FP32 = mybir.dt.float32
BF16 = mybir.dt.bfloat16
U32 = mybir.dt.uint32

BF16_NP = ml_dtypes.bfloat16


def _build_bass():
    nc = bacc.Bacc(trn_type="TRN2", debug=False, num_devices=N_CORES)

    xthi_d = nc.dram_tensor("xthi", [D, TS], BF16, kind="ExternalInput").ap()
    xtlo_d = nc.dram_tensor("xtlo", [D, TS], BF16, kind="ExternalInput").ap()
    wcat_d = nc.dram_tensor("wcat", [D, 2 * E], BF16, kind="ExternalInput").ap()
    noise_d = nc.dram_tensor("noise", [TS, E], FP32, kind="ExternalInput").ap()

    combine_d = nc.dram_tensor("combine", [TS, E], FP32, kind="ExternalOutput").ap()
    # misc packs [g1, g2, idx1(bits), idx2(bits)] per token
    misc_d = nc.dram_tensor("misc", [TS, 4], FP32, kind="ExternalOutput").ap()
    cnt_d = nc.dram_tensor("cnt", [1, E], FP32, kind="ExternalOutput").ap()

    with tile.TileContext(nc) as tc, ExitStack() as ctx:
        consts = ctx.enter_context(tc.tile_pool(name="consts", bufs=1))
        xin = ctx.enter_context(tc.tile_pool(name="xin", bufs=3))
        stsb = ctx.enter_context(tc.tile_pool(name="stsb", bufs=2))
        small = ctx.enter_context(tc.tile_pool(name="small", bufs=3))
        outp = ctx.enter_context(tc.tile_pool(name="outp", bufs=2))
        psum_st = ctx.enter_context(tc.tile_pool(name="psum_st", bufs=2, space="PSUM"))
        psum_s = ctx.enter_context(tc.tile_pool(name="psum_s", bufs=2, space="PSUM"))
        psum_c = ctx.enter_context(tc.tile_pool(name="psum_c", bufs=1, space="PSUM"))

        identity = consts.tile([P, P], FP32)
        make_identity(nc, identity)
        ones = consts.tile([P, 1], FP32)
        nc.vector.memset(ones, 1.0)

        # packed w planes [whi | wlo] as [128 d-in-chunk, 16 d-chunks, 128]
        wcat_sb = consts.tile([P, N_DC, 2 * E], BF16)
        nc.gpsimd.dma_start(
            out=wcat_sb, in_=wcat_d.rearrange("(c p) e -> p c e", p=P)
        )

        # all noise for the shard: [128, 16 chunks, 64]
        noise_sb = consts.tile([P, TS // P, E], FP32)
        nc.gpsimd.dma_start(
            out=noise_sb, in_=noise_d.rearrange("(c p) e -> p c e", p=P)
        )

        cnt_ps = psum_c.tile([1, E], FP32)

        for tb in range(N_TB):
            cols = slice(tb * TB, (tb + 1) * TB)

            # block's transposed x planes, split in d-halves so matmuls can
            # start after the first quarter of the block's data
            halves = []
            for h in range(2):
                hrows = slice(h * HALF * P, (h + 1) * HALF * P)
                th = xin.tile([P, HALF, TB], BF16, tag=f"xh{h}")
                nc.sync.dma_start(
                    out=th,
                    in_=xthi_d[hrows, cols].rearrange("(c p) t -> p c t", p=P),
                )
                tl = xin.tile([P, HALF, TB], BF16, tag=f"xl{h}")
                nc.scalar.dma_start(
                    out=tl,
                    in_=xtlo_d[hrows, cols].rearrange("(c p) t -> p c t", p=P),
                )
                halves.append((th, tl))

            # st_ps rows 0:64 = whi^T @ (xhi+xlo), rows 64:128 = wlo^T @ (xhi+xlo)
            st_ps = psum_st.tile([2 * E, TB], FP32)
            for dc in range(N_DC):
                h, c = dc // HALF, dc % HALF
                nc.tensor.matmul(
                    st_ps, lhsT=wcat_sb[:, dc, :], rhs=halves[h][0][:, c, :],
                    start=(dc == 0), stop=False, skip_group_check=True,
                )
                nc.tensor.matmul(
                    st_ps, lhsT=wcat_sb[:, dc, :], rhs=halves[h][1][:, c, :],
                    start=False, stop=(dc == N_DC - 1), skip_group_check=True,
                )

            st_sb = stsb.tile([2 * E, TB], FP32)
            for k in range(CHUNKS_PER_TB):
                ksl = slice(k * P, (k + 1) * P)
                if k % 2 == 0:
                    nc.scalar.copy(st_sb[:, ksl], st_ps[:, ksl])
                else:
                    nc.vector.tensor_copy(st_sb[:, ksl], st_ps[:, ksl])

            comb_tb = outp.tile([P, CHUNKS_PER_TB, E], FP32)
            misc_tb = outp.tile([P, CHUNKS_PER_TB, 4], FP32)

            for k in range(CHUNKS_PER_TB):
                tcv = tb * CHUNKS_PER_TB + k

                # hi/lo transposed score halves need separate PSUM tiles:
                # base-0 and base-64 matmuls into one PSUM bank fault on HW
                score_a = psum_s.tile([P, E], FP32, tag="score_a")
                score_b = psum_s.tile([P, E], FP32, tag="score_b")
                ksl = slice(k * P, (k + 1) * P)
                nc.tensor.transpose(
                    score_a, st_sb[0:E, ksl], identity[0:E, 0:E]
                )
                nc.tensor.transpose(
                    score_b, st_sb[E : 2 * E, ksl],
                    identity[E : 2 * E, E : 2 * E],
                )

                # s = scores_hi + scores_lo + noise (one PSUM operand per op)
                s_t = small.tile([P, E], FP32)
                nc.vector.tensor_add(s_t, score_a, noise_sb[:, tcv, :])
                nc.vector.tensor_add(s_t, s_t, score_b)

                top8 = small.tile([P, 8], FP32)
                nc.vector.max(out=top8, in_=s_t)
                idx8 = small.tile([P, 8], U32)
                nc.vector.max_index(out=idx8, in_max=top8, in_values=s_t)

                dv = small.tile([P, 1], FP32)
                nc.vector.tensor_sub(dv, top8[:, 0:1], top8[:, 1:2])

                misc_k = misc_tb[:, k, :]
                nc.scalar.activation(
                    misc_k[:, 0:1], dv, mybir.ActivationFunctionType.Sigmoid,
                    bias=0.0, scale=1.0,
                )
                nc.scalar.activation(
                    misc_k[:, 1:2], dv, mybir.ActivationFunctionType.Sigmoid,
                    bias=0.0, scale=-1.0,
                )
                nc.gpsimd.tensor_copy(misc_k[:, 2:4].bitcast(U32), idx8[:, 0:2])

                cmp2 = small.tile([P, E], FP32)
                nc.vector.tensor_scalar(
                    cmp2, s_t, top8[:, 1:2], None, op0=mybir.AluOpType.is_ge
                )
                cmp1 = small.tile([P, E], U32)
                nc.vector.tensor_scalar(
                    cmp1, s_t, top8[:, 0:1], None, op0=mybir.AluOpType.is_ge
                )

                # combine = cmp2 * g2, then g1 where cmp1
                comb_k = comb_tb[:, k, :]
                nc.scalar.mul(comb_k, cmp2, misc_k[:, 1:2])
                nc.vector.copy_predicated(
                    comb_k, cmp1, misc_k[:, 0:1].to_broadcast([P, E])
                )

                # expert activation counts: ones^T @ cmp2 accumulated over chunks
                nc.tensor.matmul(
                    cnt_ps, lhsT=ones, rhs=cmp2,
                    start=(tcv == 0), stop=(tcv == TS // P - 1),
                    skip_group_check=True,
                )

            nc.gpsimd.dma_start(
                out=combine_d[tb * TB : (tb + 1) * TB, :].rearrange(
                    "(c p) e -> p c e", p=P
                ),
                in_=comb_tb,
            )
            nc.gpsimd.dma_start(
                out=misc_d[tb * TB : (tb + 1) * TB, :].rearrange(
                    "(c p) k -> p c k", p=P
                ),
                in_=misc_tb,
            )

        cnt_sb = consts.tile([1, E], FP32)
        nc.vector.tensor_copy(cnt_sb, cnt_ps)
        nc.gpsimd.dma_start(out=cnt_d, in_=cnt_sb)

    nc.compile()
    return nc


_NC_CACHE = None


def kernel(x: np.ndarray, w_gate: np.ndarray, noise: np.ndarray, _trace=False):
    global _NC_CACHE
    x = np.ascontiguousarray(x, dtype=np.float32)
    w_gate = np.ascontiguousarray(w_gate, dtype=np.float32)
    noise = np.ascontiguousarray(noise, dtype=np.float32)

    # split into bf16 hi/lo planes (pseudo-fp32: hi + lo carries ~16-17
    # mantissa bits), then transpose each shard for the device layout
    xhi = x.astype(BF16_NP)
    xlo = (x - xhi.astype(np.float32)).astype(BF16_NP)
    whi = w_gate.astype(BF16_NP)
    wlo = (w_gate - whi.astype(np.float32)).astype(BF16_NP)
    wcat = np.ascontiguousarray(np.concatenate([whi, wlo], axis=1))

    in_maps = []
    for c in range(N_CORES):
        sh = slice(c * TS, (c + 1) * TS)
        in_maps.append(
            {
                "xthi": np.ascontiguousarray(xhi[sh].T),
                "xtlo": np.ascontiguousarray(xlo[sh].T),
                "wcat": wcat,
                "noise": np.ascontiguousarray(noise[sh]),
            }
        )

    if _NC_CACHE is None:
        _NC_CACHE = _build_bass()

    res = run_bass_kernel_spmd(
        _NC_CACHE, in_maps, core_ids=list(range(N_CORES)), trace=_trace
    )

    combine = np.concatenate([res.results[c]["combine"] for c in range(N_CORES)], 0)
    misc = np.concatenate([res.results[c]["misc"] for c in range(N_CORES)], 0)
    gates = np.ascontiguousarray(misc[:, 0:2])
    topk_idx = np.ascontiguousarray(misc[:, 2:4]).view(np.int32)
    expert_activation = np.sum(
        [res.results[c]["cnt"][0] for c in range(N_CORES)], axis=0, dtype=np.float32
    )

    if _trace:
        kernel._last_results = res
    return combine, topk_idx, gates, expert_activation
